# revision 10
# baseline (speedup 1.0000x reference)
"""MemoryRetriever kernel for 8x Trainium2 NeuronCores — fp8 DoubleRow version.

Data-parallel over the B*S=8192 query rows (1024 rows/core); the selected
memory bank and all weights are replicated.

Precision plan (validated vs reference, rel err ~1e-2 < 2e-2):
  - attention side (mem stats, K/V/Q projections, scores, softmax, ctx):
    fp8e4m3 operands, DoubleRow matmuls (0.5 PE cycles/row), fp32 PSUM.
  - h1 = cat @ w1.T: x-columns in bf16 (dominant error path), ctx-columns
    fp8 DoubleRow.  gate = sigmoid(cat @ gw.T): all fp8.  integ = gelu @ w2.T:
    bf16.  h1 stored bf16.  Final residual+LN in fp32.

Linear-algebra fusions (exact):
  Q = x @ (wq_in @ Wq).T + (wq_in @ bq + bqi)
  mem-layernorm folded into the K/V projections applied to RAW fp8 mem:
      K~ = mem @ (g1*wk).T - mu ck^T   (ck = rowsum of g1*wk); K = rstd * K~
      (the "mu ck^T" rank-1 term is one extra DoubleRow contraction pair)
      bk dropped: adding a constant vector to every key shifts all scores of
      a query equally -> softmax invariant.
  V likewise; bv applied to ctx after softmax (attn rows sum to 1).
  attn_out = ctx @ out_w.T + out_b folded into gate/integration weights.
"""

import sys
from contextlib import ExitStack

if "/opt/trn_rl_repo" not in sys.path:
    sys.path.insert(0, "/opt/trn_rl_repo")

import numpy as np
import ml_dtypes

import concourse.bass as bass
import concourse.mybir as mybir
import concourse.tile as tile
from concourse import bacc
from concourse.bass_utils import run_bass_kernel_spmd
from concourse.masks import make_identity

F32 = mybir.dt.float32
BF16 = mybir.dt.bfloat16
F8 = mybir.dt.float8e4
AF = mybir.ActivationFunctionType
OP = mybir.AluOpType
DR = mybir.MatmulPerfMode.DoubleRow

NP_F8 = ml_dtypes.float8_e4m3
NP_BF16 = ml_dtypes.bfloat16

H = 1024
NH = 4
HD = H // NH          # 256
K = 2048              # top_k
B, S = 4, 2048
N_CORES = 8
R = (B * S) // N_CORES  # 1024 rows per core
EPS = 1e-5
H2 = 2 * H            # 2048

HC = H // 128         # 8 feature chunks
H2C = H2 // 128       # 16
KC = K // 128         # 16 key chunks
RT = R // 512         # 2 row tiles of 512
KT4 = K // 512        # 4 key tiles of 512


def build_program():
    nc = bacc.Bacc("TRN2", target_bir_lowering=False)

    d_x32 = nc.declare_dram_parameter("x32_t", [H, R], F32, isOutput=False)
    d_x8 = nc.declare_dram_parameter("x8_t", [H, R], F8, isOutput=False)
    d_xb = nc.declare_dram_parameter("xb_t", [H, R], BF16, isOutput=False)
    d_mem8 = nc.declare_dram_parameter("mem8_t", [H, K], F8, isOutput=False)
    d_wk8 = nc.declare_dram_parameter("wk8", [HC, 128, HC, 128], F8,
                                      isOutput=False)
    d_wc8 = nc.declare_dram_parameter("wc8", [HC, 128, HC, 128], F8,
                                      isOutput=False)
    d_wvT8 = nc.declare_dram_parameter("wvT8", [128, HC, H], F8,
                                       isOutput=False)
    d_ckpad = nc.declare_dram_parameter("ckpad", [128, 2, H], F8,
                                        isOutput=False)
    d_cvpad = nc.declare_dram_parameter("cvpad", [128, 2, H], F8,
                                        isOutput=False)
    d_gw8 = nc.declare_dram_parameter("gw8", [HC, 128, H2C, 128], F8,
                                      isOutput=False)
    d_w1x = nc.declare_dram_parameter("w1x", [H2C, 128, HC, 128], BF16,
                                      isOutput=False)
    d_w1a = nc.declare_dram_parameter("w1a", [H2C, 128, HC, 128], F8,
                                      isOutput=False)
    d_w2b = nc.declare_dram_parameter("w2b", [HC, 128, H2C, 128], BF16,
                                      isOutput=False)
    d_bc = nc.declare_dram_parameter("bc", [H], F32, isOutput=False)
    d_bv = nc.declare_dram_parameter("bv", [H], F32, isOutput=False)
    d_gb = nc.declare_dram_parameter("gate_b", [H], F32, isOutput=False)
    d_b1 = nc.declare_dram_parameter("int_b1", [H2], F32, isOutput=False)
    d_b2 = nc.declare_dram_parameter("int_b2", [H], F32, isOutput=False)
    d_ilg = nc.declare_dram_parameter("iln_g", [H2], F32, isOutput=False)
    d_ilb = nc.declare_dram_parameter("iln_b", [H2], F32, isOutput=False)
    d_l2g = nc.declare_dram_parameter("ln2_g", [H], F32, isOutput=False)
    d_l2b = nc.declare_dram_parameter("ln2_b", [H], F32, isOutput=False)
    d_out = nc.declare_dram_parameter("out", [R, H], F32, isOutput=True)

    with tile.TileContext(nc) as tc, ExitStack() as top:
        singles = top.enter_context(tc.tile_pool(name="singles", bufs=1))

        ident = singles.tile([128, 128], F32)
        make_identity(nc, ident)
        scratch1 = singles.tile([128, 128], F32)
        nc.vector.memset(scratch1, 1.0)
        ones8 = singles.tile([128, 2, 128], F8)
        nc.scalar.activation(out=ones8[:, 0, :], in_=scratch1, func=AF.Copy)
        nc.scalar.activation(out=ones8[:, 1, :], in_=scratch1, func=AF.Copy)
        ones_bf = singles.tile([128, 128], BF16)
        nc.scalar.activation(out=ones_bf, in_=scratch1, func=AF.Copy)
        eps_t = singles.tile([128, 1], F32)
        nc.vector.memset(eps_t, EPS)

        def load_pp(vec, n, nm):  # [n*128] dram vector -> [128, n] per-partition
            t = singles.tile([128, n], F32, tag=f"pp_{nm}", name=f"pp_{nm}")
            nc.sync.dma_start(out=t, in_=vec[:].rearrange("(c p) -> p c", p=128))
            return t

        bc_sb = load_pp(d_bc, HC, "bc")
        bv_sb = load_pp(d_bv, HC, "bv")
        gb_sb = load_pp(d_gb, HC, "gb")
        b1_sb = load_pp(d_b1, H2C, "b1")
        b2_sb = load_pp(d_b2, HC, "b2")
        ilg_sb = load_pp(d_ilg, H2C, "ilg")
        ilb_sb = load_pp(d_ilb, H2C, "ilb")

        x8_sb = singles.tile([128, HC, R], F8)      # 1 MB, lives to D3
        for hc in range(HC):
            nc.sync.dma_start(out=x8_sb[:, hc, :],
                              in_=d_x8[hc * 128:(hc + 1) * 128, :])

        pct = top.enter_context(tc.tile_pool(name="pct", bufs=1))
        ctx8_sb = pct.tile([128, HC, R], F8)        # 1 MB, written in C
        ph1 = top.enter_context(tc.tile_pool(name="ph1", bufs=1))
        h1_sb = ph1.tile([128, H2C, R], BF16)       # 4 MB, written C..D2

        # ================== Phases B, A, C (shared scope) =================
        with ExitStack() as sabc:
            pq_sb = sabc.enter_context(tc.tile_pool(name="pq_sb", bufs=1))
            Q8_sb = pq_sb.tile([128, HC, R], F8)
            pa_keep = sabc.enter_context(tc.tile_pool(name="pa_keep", bufs=1))
            K8_sb = pa_keep.tile([128, HC, K], F8)  # 2 MB
            V8_sb = pa_keep.tile([128, KC, H], F8)  # 2 MB
            rstd_t = pa_keep.tile([128, KC], F32)
            pxb = sabc.enter_context(tc.tile_pool(name="pxb", bufs=1))
            xb_sb = pxb.tile([128, HC, R], BF16)    # 2 MB (DMA after A)

            # A tiles needed early: mem8 + squares (DVE, interleaved with B)
            pa_early = sabc.enter_context(tc.tile_pool(name="pa_early",
                                                       bufs=1))
            mem8_sb = pa_early.tile([128, HC, K], F8)   # 2 MB
            for hc in range(HC):
                nc.sync.dma_start(out=mem8_sb[:, hc, :],
                                  in_=d_mem8[hc * 128:(hc + 1) * 128, :])
            sq8 = pa_early.tile([128, HC, K], F8)       # 2 MB

            # --- B: Q projection (first on the PE, overlaps mem DMAs) ---
            with ExitStack() as sb_:
                pb_w = sb_.enter_context(tc.tile_pool(name="pb_w", bufs=3))
                pb_ps = sb_.enter_context(
                    tc.tile_pool(name="pb_ps", bufs=2, space="PSUM"))
                for oc in range(HC):
                    wcs = pb_w.tile([128, HC, 128], F8, tag="wc", name="wcs")
                    nc.sync.dma_start(out=wcs, in_=d_wc8[oc])
                    qps = [pb_ps.tile([128, 512], F32, tag=f"qps{rt}",
                                      name=f"qps{rt}") for rt in range(RT)]
                    for p in range(4):
                        for rt in range(RT):
                            nc.tensor.matmul(
                                qps[rt], wcs[:, 2 * p:2 * p + 2, :],
                                x8_sb[:, 2 * p:2 * p + 2, bass.ts(rt, 512)],
                                start=(p == 0), stop=(p == 3), perf_mode=DR)
                    nc.vector.tensor_scalar_add(out=Q8_sb[:, oc, 0:512],
                                                in0=qps[0],
                                                scalar1=bc_sb[:, oc:oc + 1])
                    nc.scalar.activation(out=Q8_sb[:, oc, 512:1024],
                                         in_=qps[1], func=AF.Identity,
                                         bias=bc_sb[:, oc:oc + 1])
                    nc.vector.tensor_mul(sq8[:, oc, :], mem8_sb[:, oc, :],
                                         mem8_sb[:, oc, :])

            # --- A: mem stats + K/V projections (mem pool scoped) ---
            with ExitStack() as sa:
                pa = sa.enter_context(tc.tile_pool(name="pa", bufs=1))
                wvT_sb = pa.tile([128, HC, H], F8)      # 1 MB
                nc.sync.dma_start(out=wvT_sb, in_=d_wvT8[:])
                ckpad_sb = pa.tile([128, 2, H], F8)
                nc.sync.dma_start(out=ckpad_sb, in_=d_ckpad[:])
                cvpad_sb = pa.tile([128, 2, H], F8)
                nc.sync.dma_start(out=cvpad_sb, in_=d_cvpad[:])
                mu_bc = pa.tile([128, K], F32)
                rstd_bc = pa.tile([128, K], F32)
                mupad = pa.tile([128, 2, K], F8)
                nc.vector.memset(mupad, 0.0)
                pa_sq = sa.enter_context(tc.tile_pool(name="pa_sq", bufs=2))
                with tc.tile_pool(name="pa_st", bufs=1, space="PSUM") as pa_st:
                    mu_ps = [pa_st.tile([128, 512], F32, tag=f"mu{i}",
                                        name=f"mu{i}") for i in range(KT4)]
                    ms_ps = [pa_st.tile([128, 512], F32, tag=f"ms{i}",
                                        name=f"ms{i}") for i in range(KT4)]
                    for p in range(4):
                        for i in range(KT4):
                            nc.tensor.matmul(
                                mu_ps[i], ones8,
                                mem8_sb[:, 2 * p:2 * p + 2, bass.ts(i, 512)],
                                start=(p == 0), stop=(p == 3), perf_mode=DR)
                    for p in range(4):
                        for i in range(KT4):
                            nc.tensor.matmul(
                                ms_ps[i], ones8,
                                sq8[:, 2 * p:2 * p + 2, bass.ts(i, 512)],
                                start=(p == 0), stop=(p == 3), perf_mode=DR)
                    for i in range(KT4):
                        sl = bass.ts(i, 512)
                        nc.scalar.activation(out=mu_bc[:, sl], in_=mu_ps[i],
                                             func=AF.Copy, scale=1.0 / 1024.0)
                        msq = pa_sq.tile([128, 512], F32, tag="msq",
                                         name="msq")
                        nc.vector.tensor_mul(msq, mu_bc[:, sl], mu_bc[:, sl])
                        var = pa_sq.tile([128, 512], F32, tag="var",
                                         name="var")
                        nc.vector.scalar_tensor_tensor(
                            out=var, in0=ms_ps[i], scalar=1.0 / 1024.0,
                            in1=msq, op0=OP.mult, op1=OP.subtract)
                        nc.scalar.activation(out=var, in_=var, func=AF.Ln,
                                             bias=eps_t, scale=1.0)
                        nc.scalar.activation(out=rstd_bc[:, sl], in_=var,
                                             func=AF.Exp, scale=-0.5)
                # mupad row 0 half 0 = fp8(mu)
                nc.scalar.activation(out=mupad[0:1, 0, :], in_=mu_bc[0:1, :],
                                     func=AF.Copy)

                with ExitStack() as skv:
                    pa_w = skv.enter_context(tc.tile_pool(name="pa_w",
                                                          bufs=3))
                    pa_ps = skv.enter_context(
                        tc.tile_pool(name="pa_ps", bufs=2, space="PSUM"))
                    # K projection -> feature-major K8, rstd folded at drain
                    for oc in range(HC):
                        kps = [pa_ps.tile([128, 512], F32, tag=f"kps{i}",
                                          name=f"kps{i}") for i in range(KT4)]
                        wks = pa_w.tile([128, HC, 128], F8, tag="wk",
                                        name="wks")
                        nc.sync.dma_start(out=wks, in_=d_wk8[oc])
                        for p in range(4):
                            for i in range(KT4):
                                nc.tensor.matmul(
                                    kps[i], wks[:, 2 * p:2 * p + 2, :],
                                    mem8_sb[:, 2 * p:2 * p + 2,
                                            bass.ts(i, 512)],
                                    start=(p == 0), stop=False, perf_mode=DR)
                        for i in range(KT4):
                            nc.tensor.matmul(
                                kps[i],
                                ckpad_sb[:, :, oc * 128:(oc + 1) * 128],
                                mupad[:, :, bass.ts(i, 512)],
                                start=False, stop=True, perf_mode=DR)
                        for i in range(KT4):
                            sl = bass.ts(i, 512)
                            nc.vector.tensor_mul(K8_sb[:, oc, sl], kps[i],
                                                 rstd_bc[:, sl])
                    # transposed rstd (per-key partitions) for the V drain
                    for kc in range(KC):
                        tpp = pa_ps.tile([128, 512], F32, tag=f"kps{kc % 2}",
                                         name="tpp")
                        nc.tensor.transpose(
                            tpp[:, 0:128], rstd_bc[:, kc * 128:(kc + 1) * 128],
                            ident)
                        nc.scalar.activation(out=rstd_t[:, kc:kc + 1],
                                             in_=tpp[:, 0:1], func=AF.Copy)
                    # V projection -> key-major V8 [128, KC, H]
                    for kc in range(KC):
                        vps = [pa_ps.tile([128, 512], F32, tag=f"kps{2 + ow}",
                                          name=f"vps{ow}")
                               for ow in range(2)]
                        for p in range(4):
                            for ow in range(2):
                                nc.tensor.matmul(
                                    vps[ow],
                                    mem8_sb[:, 2 * p:2 * p + 2,
                                            kc * 128:(kc + 1) * 128],
                                    wvT_sb[:, 2 * p:2 * p + 2,
                                           bass.ts(ow, 512)],
                                    start=(p == 0), stop=False, perf_mode=DR)
                        for ow in range(2):
                            nc.tensor.matmul(
                                vps[ow], mupad[:, :, kc * 128:(kc + 1) * 128],
                                cvpad_sb[:, :, bass.ts(ow, 512)],
                                start=False, stop=True, perf_mode=DR)
                        nc.vector.tensor_scalar_mul(
                            out=V8_sb[:, kc, 0:512], in0=vps[0],
                            scalar1=rstd_t[:, kc:kc + 1])
                        nc.scalar.activation(out=V8_sb[:, kc, 512:1024],
                                             in_=vps[1], func=AF.Copy,
                                             scale=rstd_t[:, kc:kc + 1])

            # xb for the interleaved D1 x-part (DMA lands during late A)
            for hc in range(HC):
                nc.sync.dma_start(out=xb_sb[:, hc, :],
                                  in_=d_xb[hc * 128:(hc + 1) * 128, :])

            # --- C: attention (scalar-bound) interleaved with D1 x-part ---
            with ExitStack() as sc_:
                pc_e = sc_.enter_context(tc.tile_pool(name="pc_e", bufs=1))
                pc_o = sc_.enter_context(tc.tile_pool(name="pc_o", bufs=2))
                pw1x = sc_.enter_context(tc.tile_pool(name="pw1x", bufs=3))
                pc_ps = sc_.enter_context(
                    tc.tile_pool(name="pc_ps", bufs=1, space="PSUM"))
                for h in range(NH):
                    e2 = [[pc_e.tile([128, 2, 512], F8, tag=f"e{tp}_{qt}",
                                     name=f"e{tp}_{qt}") for qt in range(RT)]
                          for tp in range(8)]
                    ctx_ps = [[pc_ps.tile([128, 512], F32, tag=f"ctx{qt}{j}",
                                          name=f"ctx{qt}{j}")
                               for j in range(2)] for qt in range(RT)]
                    for tp in range(8):
                        for ktl in range(2):
                            kt = 2 * tp + ktl
                            scs = [pc_ps.tile([128, 512], F32, tag=f"sc{qt}",
                                              name=f"sc{qt}")
                                   for qt in range(RT)]
                            for qt in range(RT):
                                nc.tensor.matmul(
                                    scs[qt],
                                    K8_sb[:, 2 * h:2 * h + 2,
                                          kt * 128:(kt + 1) * 128],
                                    Q8_sb[:, 2 * h:2 * h + 2,
                                          bass.ts(qt, 512)],
                                    start=True, stop=True, perf_mode=DR)
                            for qt in range(RT):
                                nc.scalar.activation(
                                    out=e2[tp][qt][:, ktl, :], in_=scs[qt],
                                    func=AF.Exp, scale=1.0 / 16.0)
                        for j in range(2):
                            for qt in range(RT):
                                nc.tensor.matmul(
                                    ctx_ps[qt][j],
                                    V8_sb[:, 2 * tp:2 * tp + 2,
                                          h * HD + j * 128:
                                          h * HD + (j + 1) * 128],
                                    e2[tp][qt],
                                    start=(tp == 0), stop=(tp == 7),
                                    perf_mode=DR)
                    # deferred softmax sums (single ones LDW, reuses sc banks)
                    sums_ps = [pc_ps.tile([128, 512], F32, tag=f"sc{qt}",
                                          name=f"sums{qt}")
                               for qt in range(RT)]
                    for tp in range(8):
                        for qt in range(RT):
                            nc.tensor.matmul(
                                sums_ps[qt], ones8, e2[tp][qt],
                                start=(tp == 0), stop=(tp == 7), perf_mode=DR)
                    for qt in range(RT):
                        qsl = bass.ts(qt, 512)
                        rec = pc_o.tile([128, 512], F32, tag="rec",
                                        name="rec")
                        nc.vector.reciprocal_approx_fast(out=rec,
                                                         in_=sums_ps[qt])
                        for j in range(2):
                            tmp = pc_o.tile([128, 512], F32, tag="ctmp",
                                            name="ctmp")
                            nc.vector.tensor_mul(tmp, ctx_ps[qt][j], rec)
                            nc.scalar.activation(
                                out=ctx8_sb[:, 2 * h + j, qsl], in_=tmp,
                                func=AF.Identity,
                                bias=bv_sb[:, 2 * h + j:2 * h + j + 1])
                    # D1 x-part: 4 output chunks per head iteration
                    for oc2 in range(4 * h, 4 * h + 4):
                        w1xs = pw1x.tile([128, HC, 128], BF16, tag="w1x",
                                         name="w1xs")
                        nc.sync.dma_start(out=w1xs, in_=d_w1x[oc2])
                        xps = [pc_ps.tile([128, 512], F32, tag=f"xps{rt}",
                                          name=f"xps{rt}")
                               for rt in range(RT)]
                        for hc in range(HC):
                            for rt in range(RT):
                                nc.tensor.matmul(
                                    xps[rt], w1xs[:, hc, :],
                                    xb_sb[:, hc, bass.ts(rt, 512)],
                                    start=(hc == 0), stop=(hc == HC - 1))
                        for rt in range(RT):
                            nc.vector.tensor_scalar_add(
                                out=h1_sb[:, oc2, bass.ts(rt, 512)],
                                in0=xps[rt],
                                scalar1=b1_sb[:, oc2:oc2 + 1])

        # ================= Phase D: gated integration MLP =================
        l2g_bc = singles.tile([128, H], F32)
        nc.sync.dma_start(
            out=l2g_bc,
            in_=d_l2g[:].unsqueeze(0).partition_broadcast(128).squeeze(1))
        l2b_bc = singles.tile([128, H], F32)
        nc.sync.dma_start(
            out=l2b_bc,
            in_=d_l2b[:].unsqueeze(0).partition_broadcast(128).squeeze(1))

        with ExitStack() as sd:
            pd = sd.enter_context(tc.tile_pool(name="pd", bufs=1))
            x32_sb = pd.tile([128, HC, R], F32)    # 4 MB (used in D3)

            with ExitStack() as sd12:
                pd_st = sd12.enter_context(tc.tile_pool(name="pd_st", bufs=1))
                mu2_sb = pd_st.tile([128, R], F32)
                rstd2_sb = pd_st.tile([128, R], F32)
                pd_sq = sd12.enter_context(tc.tile_pool(name="pd_sq", bufs=2))
                pd_w1 = sd12.enter_context(tc.tile_pool(name="pd_w1", bufs=3))
                pd_ps = sd12.enter_context(
                    tc.tile_pool(name="pd_ps", bufs=2, space="PSUM"))
                pd_ps2 = sd12.enter_context(
                    tc.tile_pool(name="pd_ps2", bufs=1, space="PSUM"))
                mu2_ps = [pd_ps2.tile([128, 512], F32, tag=f"m2_{i}",
                                      name=f"m2_{i}") for i in range(RT)]
                ms2_ps = [pd_ps2.tile([128, 512], F32, tag=f"s2_{i}",
                                      name=f"s2_{i}") for i in range(RT)]
                sq2s = [None] * H2C

                def d2_stats(oc2):
                    for rt in range(RT):
                        sl = bass.ts(rt, 512)
                        nc.tensor.matmul(mu2_ps[rt], ones_bf,
                                         h1_sb[:, oc2, sl],
                                         start=(oc2 == 0),
                                         stop=(oc2 == H2C - 1))
                        nc.tensor.matmul(ms2_ps[rt], ones_bf,
                                         sq2s[oc2][:, sl],
                                         start=(oc2 == 0),
                                         stop=(oc2 == H2C - 1))

                # D1 remainder: h1 += ctx-part (fp8 DR), then D2 stats
                for oc2 in range(H2C):
                    w1as = pd_w1.tile([128, HC, 128], F8, tag="w1a",
                                      name="w1as")
                    nc.sync.dma_start(out=w1as, in_=d_w1a[oc2])
                    cps = [pd_ps.tile([128, 512], F32, tag=f"cps{rt}",
                                      name=f"cps{rt}") for rt in range(RT)]
                    for p in range(4):
                        for rt in range(RT):
                            nc.tensor.matmul(
                                cps[rt], w1as[:, 2 * p:2 * p + 2, :],
                                ctx8_sb[:, 2 * p:2 * p + 2, bass.ts(rt, 512)],
                                start=(p == 0), stop=(p == 3), perf_mode=DR)
                    for rt in range(RT):
                        sl = bass.ts(rt, 512)
                        nc.vector.tensor_add(h1_sb[:, oc2, sl], cps[rt],
                                             h1_sb[:, oc2, sl])
                    sq2 = pd_sq.tile([128, R], BF16, tag="sq2",
                                     name="sq2", bufs=H2C)
                    nc.scalar.activation(out=sq2, in_=h1_sb[:, oc2, :],
                                         func=AF.Square)
                    sq2s[oc2] = sq2
                # D2 stats as one uninterrupted PE pass
                for oc2 in range(H2C):
                    d2_stats(oc2)

                # x32 lands during the D2 tail window
                for hc in range(HC):
                    nc.sync.dma_start(out=x32_sb[:, hc, :],
                                      in_=d_x32[hc * 128:(hc + 1) * 128, :])

                # D2 tail: mu2/rstd2, then LN-apply + exact gelu in place
                for rt in range(RT):
                    sl = bass.ts(rt, 512)
                    nc.scalar.activation(out=mu2_sb[:, sl], in_=mu2_ps[rt],
                                         func=AF.Copy, scale=1.0 / 2048.0)
                    msq = pd_sq.tile([128, 512], F32, tag="msq2", name="msq2")
                    nc.vector.tensor_mul(msq, mu2_sb[:, sl], mu2_sb[:, sl])
                    var = pd_sq.tile([128, 512], F32, tag="var2", name="var2")
                    nc.vector.scalar_tensor_tensor(
                        out=var, in0=ms2_ps[rt], scalar=1.0 / 2048.0,
                        in1=msq, op0=OP.mult, op1=OP.subtract)
                    nc.scalar.activation(out=var, in_=var, func=AF.Ln,
                                         bias=eps_t, scale=1.0)
                    nc.scalar.activation(out=rstd2_sb[:, sl], in_=var,
                                         func=AF.Exp, scale=-0.5)
                for rt in range(RT):
                    sl = bass.ts(rt, 512)
                    for oc2 in range(H2C):
                        t1 = pd_sq.tile([128, 512], F32, tag="t1d",
                                        name="t1d")
                        nc.vector.tensor_sub(t1, h1_sb[:, oc2, sl],
                                             mu2_sb[:, sl])
                        nc.vector.scalar_tensor_tensor(
                            out=t1, in0=t1, scalar=ilg_sb[:, oc2:oc2 + 1],
                            in1=rstd2_sb[:, sl], op0=OP.mult, op1=OP.mult)
                        nc.scalar.activation(out=h1_sb[:, oc2, sl], in_=t1,
                                             func=AF.Gelu,
                                             bias=ilb_sb[:, oc2:oc2 + 1])

            # D3: gate = sigmoid(cat@gw.T+gb) [fp8]; integ = gelu@w2.T + b2
            #     y = x + gate*integ (fp32); D4: final row LN via transposes
            with ExitStack() as sd34:
                pd_wd3 = sd34.enter_context(tc.tile_pool(name="pd_wd3",
                                                         bufs=3))
                pd_y = sd34.enter_context(tc.tile_pool(name="pd_y", bufs=1))
                yt_sb = pd_y.tile([128, HC, R], F32)
                pd_o = sd34.enter_context(tc.tile_pool(name="pd_o", bufs=2))
                pd_yr = sd34.enter_context(tc.tile_pool(name="pd_yr",
                                                        bufs=4))
                pd_ps3 = sd34.enter_context(
                    tc.tile_pool(name="pd_ps3", bufs=2, space="PSUM"))
                pd_ps4 = sd34.enter_context(
                    tc.tile_pool(name="pd_ps4", bufs=2, space="PSUM"))

                def cat8_pair(p, sl):
                    if p < 4:
                        return x8_sb[:, 2 * p:2 * p + 2, sl]
                    return ctx8_sb[:, 2 * (p - 4):2 * (p - 4) + 2, sl]

                all_sigs = {}

                def gates(rt):
                    sl = bass.ts(rt, 512)
                    sigs = []
                    for oc in range(HC):
                        gws = pd_wd3.tile([128, H2C, 128], F8, tag="gw",
                                          name="gws")
                        nc.sync.dma_start(out=gws, in_=d_gw8[oc])
                        gps = pd_ps3.tile([128, 512], F32, tag="gps",
                                          name="gps")
                        for p in range(8):
                            nc.tensor.matmul(
                                gps, gws[:, 2 * p:2 * p + 2, :],
                                cat8_pair(p, sl),
                                start=(p == 0), stop=(p == 7), perf_mode=DR)
                        sig = pd_o.tile([128, 512], BF16, tag="sig",
                                        name="sig", bufs=8)
                        nc.scalar.activation(out=sig, in_=gps,
                                             func=AF.Sigmoid,
                                             bias=gb_sb[:, oc:oc + 1])
                        sigs.append(sig)
                    all_sigs[rt] = sigs

                def integs(rt):
                    sl = bass.ts(rt, 512)
                    for oc in range(HC):
                        w2s = pd_wd3.tile([128, H2C, 128], BF16, tag="w2",
                                          name="w2s")
                        nc.sync.dma_start(out=w2s, in_=d_w2b[oc])
                        igps = pd_ps3.tile([128, 512], F32, tag="igps",
                                           name="igps")
                        for hc in range(H2C):
                            nc.tensor.matmul(igps, w2s[:, hc, :],
                                             h1_sb[:, hc, sl],
                                             start=(hc == 0),
                                             stop=(hc == H2C - 1))
                        tmp = pd_o.tile([128, 512], F32, tag="ytmp",
                                        name="ytmp")
                        nc.vector.scalar_tensor_tensor(
                            out=tmp, in0=igps, scalar=b2_sb[:, oc:oc + 1],
                            in1=all_sigs[rt][oc], op0=OP.add, op1=OP.mult)
                        nc.vector.tensor_add(yt_sb[:, oc, sl], tmp,
                                             x32_sb[:, oc, sl])

                def d4_pass(rt):
                    mvs = pd_o.tile([128, 4, 2], F32, tag="mvs", name="mvs")
                    yrs = []
                    for i, rc in enumerate(range(rt * 4, rt * 4 + 4)):
                        tpp = pd_ps4.tile([128, 1024], F32, tag="tp",
                                          name="tp")
                        for oc in range(HC):
                            nc.tensor.transpose(
                                tpp[:, oc * 128:(oc + 1) * 128],
                                yt_sb[:, oc, rc * 128:(rc + 1) * 128], ident)
                        yr = pd_yr.tile([128, H], F32, tag="yr", name="yr")
                        nc.scalar.activation(out=yr[:, 0:512],
                                             in_=tpp[:, 0:512], func=AF.Copy)
                        nc.scalar.activation(out=yr[:, 512:1024],
                                             in_=tpp[:, 512:1024],
                                             func=AF.Copy)
                        stats = pd_o.tile([128, 2, 6], F32, tag="bst",
                                          name="bst")
                        for k in range(2):
                            nc.vector.bn_stats(
                                out=stats[:, k, :],
                                in_=yr[:, k * 512:(k + 1) * 512])
                        nc.vector.bn_aggr(out=mvs[:, i, :], in_=stats)
                        yrs.append(yr)
                    rstds = pd_o.tile([128, 4], F32, tag="rstds",
                                      name="rstds")
                    nc.scalar.activation(out=rstds, in_=mvs[:, :, 1],
                                         func=AF.Ln, bias=eps_t, scale=1.0)
                    nc.scalar.activation(out=rstds, in_=rstds,
                                         func=AF.Exp, scale=-0.5)
                    nmrs = pd_o.tile([128, 4], F32, tag="nmrs", name="nmrs")
                    nc.vector.scalar_tensor_tensor(
                        out=nmrs, in0=mvs[:, :, 0], scalar=-1.0, in1=rstds,
                        op0=OP.mult, op1=OP.mult)
                    for i, rc in enumerate(range(rt * 4, rt * 4 + 4)):
                        yr = yrs[i]
                        nc.scalar.activation(out=yr, in_=yr, func=AF.Identity,
                                             bias=nmrs[:, i:i + 1],
                                             scale=rstds[:, i:i + 1])
                        nc.vector.tensor_mul(yr, yr, l2g_bc)
                        nc.vector.tensor_add(yr, yr, l2b_bc)
                        nc.sync.dma_start(
                            out=d_out[rc * 128:(rc + 1) * 128, :], in_=yr)

                gates(0)
                integs(0)
                gates(1)
                d4_pass(0)
                integs(1)
                d4_pass(1)

    nc.compile()
    return nc


_NC_CACHE = []


def _get_nc():
    if not _NC_CACHE:
        _NC_CACHE.append(build_program())
    return _NC_CACHE[0]


def kernel(query_hidden, mem_keys, importance, recency, access_count,
           Wq, bq, in_w, in_b, out_w, out_b, gate_w, gate_b,
           int_w1, int_b1, int_ln_g, int_ln_b, int_w2, int_b2,
           ln1_g, ln1_b, ln2_g, ln2_b, sel_params, top_k):
    np32 = lambda a: np.asarray(a, dtype=np.float32)
    f8 = lambda a: np.ascontiguousarray(np.asarray(a, np.float32).astype(NP_F8))
    bf = lambda a: np.ascontiguousarray(np.asarray(a, np.float32).astype(NP_BF16))
    query_hidden = np32(query_hidden)
    mem_keys = np32(mem_keys)
    top_k = int(top_k)
    assert top_k == K, f"kernel compiled for top_k={K}, got {top_k}"

    # HTPS selection (host): softmax-weighted score, top-k set, gather.
    # Attention output is invariant to the order of the selected rows.
    sp = np32(sel_params)
    w = np.exp(sp - sp.max())
    w = w / w.sum()
    acc = np32(access_count)
    sel = w[0] * np32(importance) + w[1] * np32(recency) + w[2] * (acc / acc.max())
    idx = np.argpartition(-sel, top_k - 1)[:top_k]
    mem = mem_keys[idx]                                 # [K, H]

    in_w = np32(in_w)
    in_b = np32(in_b)
    wq, wk, wv = in_w[:H], in_w[H:2 * H], in_w[2 * H:]
    bqi, bki, bvi = in_b[:H], in_b[H:2 * H], in_b[2 * H:]
    wc = wq @ np32(Wq)                                  # fused query projection
    bc = wq @ np32(bq) + bqi

    # fold mem-layernorm gamma into wk/wv; beta into the biases.
    # bk is softmax-invariant -> dropped. bv applied to ctx post-softmax.
    g1 = np32(ln1_g)
    b1v = np32(ln1_b)
    bv_f = bvi + wv @ b1v
    wk_f = wk * g1[None, :]
    wv_f = wv * g1[None, :]
    ck = wk_f.sum(axis=1)                               # [H]
    cv = wv_f.sum(axis=1)

    ckpad = np.zeros((128, 2, H), NP_F8)
    ckpad[0, 0, :] = (-ck).astype(NP_F8)
    cvpad = np.zeros((128, 2, H), NP_F8)
    cvpad[0, 0, :] = (-cv).astype(NP_F8)

    # fold attn_out = ctx @ out_w.T + out_b into gate / integration weights
    out_w = np32(out_w)
    out_b = np32(out_b)
    gate_w = np32(gate_w)
    int_w1 = np32(int_w1)
    gwx, gwa = gate_w[:, :H], gate_w[:, H:]
    w1x, w1a = int_w1[:, :H], int_w1[:, H:]
    gate_b_f = np32(gate_b) + gwa @ out_b
    int_b1_f = np32(int_b1) + w1a @ out_b

    T = lambda a: np.ascontiguousarray(np32(a).T)

    def chunked(w_t, dt):
        # [IN, OUT] -> [OUT//128, 128, IN//128, 128] contiguous slabs
        inn, out = w_t.shape
        r = w_t.reshape(inn // 128, 128, out // 128, 128).transpose(2, 1, 0, 3)
        return np.ascontiguousarray(r.astype(dt))

    gw_full = np.concatenate([gwx.T, (gwa @ out_w).T], axis=0)   # [2H, H]
    wvT = np.ascontiguousarray(
        wv_f.T.reshape(HC, 128, H).transpose(1, 0, 2))           # [128, HC, H]

    common = {
        "mem8_t": f8(mem.T),
        "wk8": chunked(T(wk_f), NP_F8),
        "wc8": chunked(T(wc), NP_F8),
        "wvT8": wvT.astype(NP_F8),
        "ckpad": ckpad, "cvpad": cvpad,
        "gw8": chunked(gw_full, NP_F8),
        "w1x": chunked(T(w1x), NP_BF16),
        "w1a": chunked(T(w1a @ out_w), NP_F8),
        "w2b": chunked(T(np32(int_w2)), NP_BF16),
        "bc": bc, "bv": bv_f,
        "gate_b": gate_b_f, "int_b1": int_b1_f, "int_b2": np32(int_b2),
        "iln_g": np32(int_ln_g), "iln_b": np32(int_ln_b),
        "ln2_g": np32(ln2_g), "ln2_b": np32(ln2_b),
    }
    X = query_hidden.reshape(B * S, H)
    in_maps = []
    for c in range(N_CORES):
        xt = np.ascontiguousarray(X[c * R:(c + 1) * R].T)
        m = dict(common)
        m["x32_t"] = xt
        m["x8_t"] = f8(xt)
        m["xb_t"] = bf(xt)
        in_maps.append(m)

    nc = _get_nc()
    res = run_bass_kernel_spmd(nc, in_maps, core_ids=list(range(N_CORES)))
    out = np.empty((B * S, H), dtype=np.float32)
    for c in range(N_CORES):
        out[c * R:(c + 1) * R] = res.results[c]["out"]
    return out.reshape(B, S, H)


# revision 13
# speedup vs baseline: 1.1072x; 1.1072x over previous
"""MemoryRetriever kernel for 8x Trainium2 NeuronCores — fp8 DoubleRow version.

Data-parallel over the B*S=8192 query rows (1024 rows/core); the selected
memory bank and all weights are replicated.

Precision plan (validated vs reference, rel err ~1e-2 < 2e-2):
  - attention side (mem stats, K/V/Q projections, scores, softmax, ctx):
    fp8e4m3 operands, DoubleRow matmuls (0.5 PE cycles/row), fp32 PSUM.
  - h1 = cat @ w1.T: x-columns in bf16 (dominant error path), ctx-columns
    fp8 DoubleRow.  gate = sigmoid(cat @ gw.T): all fp8.  integ = gelu @ w2.T:
    bf16.  h1 stored bf16.  Final residual+LN in fp32.

Linear-algebra fusions (exact):
  Q = x @ (wq_in @ Wq).T + (wq_in @ bq + bqi)
  mem-layernorm folded into the K/V projections applied to RAW fp8 mem:
      K~ = mem @ (g1*wk).T - mu ck^T   (ck = rowsum of g1*wk); K = rstd * K~
      (the "mu ck^T" rank-1 term is one extra DoubleRow contraction pair)
      bk dropped: adding a constant vector to every key shifts all scores of
      a query equally -> softmax invariant.
  V likewise; bv applied to ctx after softmax (attn rows sum to 1).
  attn_out = ctx @ out_w.T + out_b folded into gate/integration weights.
"""

import sys
from contextlib import ExitStack

if "/opt/trn_rl_repo" not in sys.path:
    sys.path.insert(0, "/opt/trn_rl_repo")

import numpy as np
import ml_dtypes

import concourse.bass as bass
import concourse.mybir as mybir
import concourse.tile as tile
from concourse import bacc
from concourse.bass_utils import run_bass_kernel_spmd
from concourse.masks import make_identity

F32 = mybir.dt.float32
BF16 = mybir.dt.bfloat16
F8 = mybir.dt.float8e4
AF = mybir.ActivationFunctionType
OP = mybir.AluOpType
DR = mybir.MatmulPerfMode.DoubleRow

NP_F8 = ml_dtypes.float8_e4m3
NP_BF16 = ml_dtypes.bfloat16

H = 1024
NH = 4
HD = H // NH          # 256
K = 2048              # top_k
B, S = 4, 2048
N_CORES = 8
R = (B * S) // N_CORES  # 1024 rows per core
EPS = 1e-5
H2 = 2 * H            # 2048

HC = H // 128         # 8 feature chunks
H2C = H2 // 128       # 16
KC = K // 128         # 16 key chunks
RT = R // 512         # 2 row tiles of 512
KT4 = K // 512        # 4 key tiles of 512


def build_program():
    nc = bacc.Bacc("TRN2", target_bir_lowering=False)

    d_x32 = nc.declare_dram_parameter("x32_t", [H, R], F32, isOutput=False)
    d_x8 = nc.declare_dram_parameter("x8_t", [H, R], F8, isOutput=False)
    d_xb = nc.declare_dram_parameter("xb_t", [H, R], BF16, isOutput=False)
    d_mem8 = nc.declare_dram_parameter("mem8_t", [H, K], F8, isOutput=False)
    d_wk8 = nc.declare_dram_parameter("wk8", [HC, 128, HC, 128], F8,
                                      isOutput=False)
    d_wc8 = nc.declare_dram_parameter("wc8", [HC, 128, HC, 128], F8,
                                      isOutput=False)
    d_wvT8 = nc.declare_dram_parameter("wvT8", [128, HC, H], F8,
                                       isOutput=False)
    d_ckpad = nc.declare_dram_parameter("ckpad", [128, 2, H], F8,
                                        isOutput=False)
    d_cvpad = nc.declare_dram_parameter("cvpad", [128, 2, H], F8,
                                        isOutput=False)
    d_gw8 = nc.declare_dram_parameter("gw8", [HC, 128, H2C, 128], F8,
                                      isOutput=False)
    d_w1x = nc.declare_dram_parameter("w1x", [H2C, 128, HC, 128], BF16,
                                      isOutput=False)
    d_w1a = nc.declare_dram_parameter("w1a", [H2C, 128, HC, 128], F8,
                                      isOutput=False)
    d_w2b = nc.declare_dram_parameter("w2b", [HC, 128, H2C, 128], BF16,
                                      isOutput=False)
    d_bc = nc.declare_dram_parameter("bc", [H], F32, isOutput=False)
    d_bv = nc.declare_dram_parameter("bv", [H], F32, isOutput=False)
    d_gb = nc.declare_dram_parameter("gate_b", [H], F32, isOutput=False)
    d_b1 = nc.declare_dram_parameter("int_b1", [H2], F32, isOutput=False)
    d_b2 = nc.declare_dram_parameter("int_b2", [H], F32, isOutput=False)
    d_ilg = nc.declare_dram_parameter("iln_g", [H2], F32, isOutput=False)
    d_ilb = nc.declare_dram_parameter("iln_b", [H2], F32, isOutput=False)
    d_l2g = nc.declare_dram_parameter("ln2_g", [H], F32, isOutput=False)
    d_l2b = nc.declare_dram_parameter("ln2_b", [H], F32, isOutput=False)
    d_out = nc.declare_dram_parameter("out", [R, H], F32, isOutput=True)

    with tile.TileContext(nc) as tc, ExitStack() as top:
        singles = top.enter_context(tc.tile_pool(name="singles", bufs=1))

        ident = singles.tile([128, 128], F32)
        make_identity(nc, ident)
        scratch1 = singles.tile([128, 128], F32)
        nc.vector.memset(scratch1, 1.0)
        ones8 = singles.tile([128, 2, 128], F8)
        nc.scalar.activation(out=ones8[:, 0, :], in_=scratch1, func=AF.Copy)
        nc.scalar.activation(out=ones8[:, 1, :], in_=scratch1, func=AF.Copy)
        ones_bf = singles.tile([128, 128], BF16)
        nc.scalar.activation(out=ones_bf, in_=scratch1, func=AF.Copy)
        eps_t = singles.tile([128, 1], F32)
        nc.vector.memset(eps_t, EPS)

        def load_pp(vec, n, nm):  # [n*128] dram vector -> [128, n] per-partition
            t = singles.tile([128, n], F32, tag=f"pp_{nm}", name=f"pp_{nm}")
            nc.sync.dma_start(out=t, in_=vec[:].rearrange("(c p) -> p c", p=128))
            return t

        bc_sb = load_pp(d_bc, HC, "bc")
        bv_sb = load_pp(d_bv, HC, "bv")
        gb_sb = load_pp(d_gb, HC, "gb")
        b1_sb = load_pp(d_b1, H2C, "b1")
        b2_sb = load_pp(d_b2, HC, "b2")
        ilg_sb = load_pp(d_ilg, H2C, "ilg")
        ilb_sb = load_pp(d_ilb, H2C, "ilb")

        x8_sb = singles.tile([128, HC, R], F8)      # 1 MB, lives to D3
        for hc in range(HC):
            nc.sync.dma_start(out=x8_sb[:, hc, :],
                              in_=d_x8[hc * 128:(hc + 1) * 128, :])

        pct = top.enter_context(tc.tile_pool(name="pct", bufs=1))
        ctx8_sb = pct.tile([128, HC, R], F8)        # 1 MB, written in C
        ph1 = top.enter_context(tc.tile_pool(name="ph1", bufs=1))
        h1_sb = ph1.tile([128, H2C, R], BF16)       # 4 MB, written C..D2

        # ================== Phases B, A, C (shared scope) =================
        with ExitStack() as sabc:
            pq_sb = sabc.enter_context(tc.tile_pool(name="pq_sb", bufs=1))
            Q8_sb = pq_sb.tile([128, HC, R], F8)
            pa_keep = sabc.enter_context(tc.tile_pool(name="pa_keep", bufs=1))
            K8_sb = pa_keep.tile([128, HC, K], F8)  # 2 MB
            V8_sb = pa_keep.tile([128, KC, H], F8)  # 2 MB
            rstd_t = pa_keep.tile([128, KC], F32)
            rstd16_t = pa_keep.tile([128, KC], F32)
            pxb = sabc.enter_context(tc.tile_pool(name="pxb", bufs=1))
            xb_sb = pxb.tile([128, HC, R], BF16)    # 2 MB (DMA after A)

            # A tiles needed early: mem8 + squares (DVE, interleaved with B)
            pa_early = sabc.enter_context(tc.tile_pool(name="pa_early",
                                                       bufs=1))
            mem8_sb = pa_early.tile([128, HC, K], F8)   # 2 MB
            for hc in range(HC):
                nc.sync.dma_start(out=mem8_sb[:, hc, :],
                                  in_=d_mem8[hc * 128:(hc + 1) * 128, :])
            sq8 = pa_early.tile([128, HC, K], F8)       # 2 MB

            # --- B: Q projection (first on the PE, overlaps mem DMAs) ---
            with ExitStack() as sb_:
                pb_w = sb_.enter_context(tc.tile_pool(name="pb_w", bufs=3))
                pb_ps = sb_.enter_context(
                    tc.tile_pool(name="pb_ps", bufs=2, space="PSUM"))
                for oc in range(HC):
                    wcs = pb_w.tile([128, HC, 128], F8, tag="wc", name="wcs")
                    nc.sync.dma_start(out=wcs, in_=d_wc8[oc])
                    qps = [pb_ps.tile([128, 512], F32, tag=f"qps{rt}",
                                      name=f"qps{rt}") for rt in range(RT)]
                    for p in range(4):
                        for rt in range(RT):
                            nc.tensor.matmul(
                                qps[rt], wcs[:, 2 * p:2 * p + 2, :],
                                x8_sb[:, 2 * p:2 * p + 2, bass.ts(rt, 512)],
                                start=(p == 0), stop=(p == 3), perf_mode=DR)
                    nc.vector.tensor_scalar_add(out=Q8_sb[:, oc, 0:512],
                                                in0=qps[0],
                                                scalar1=bc_sb[:, oc:oc + 1])
                    nc.scalar.activation(out=Q8_sb[:, oc, 512:1024],
                                         in_=qps[1], func=AF.Identity,
                                         bias=bc_sb[:, oc:oc + 1])
                    nc.vector.tensor_mul(sq8[:, oc, :], mem8_sb[:, oc, :],
                                         mem8_sb[:, oc, :])

            # --- A: mem stats + K/V projections (mem pool scoped) ---
            with ExitStack() as sa:
                pa = sa.enter_context(tc.tile_pool(name="pa", bufs=1))
                wvT_sb = pa.tile([128, HC, H], F8)      # 1 MB
                nc.sync.dma_start(out=wvT_sb, in_=d_wvT8[:])
                ckpad_sb = pa.tile([128, 2, H], F8)
                nc.sync.dma_start(out=ckpad_sb, in_=d_ckpad[:])
                cvpad_sb = pa.tile([128, 2, H], F8)
                nc.sync.dma_start(out=cvpad_sb, in_=d_cvpad[:])
                mu_bc = pa.tile([128, K], F32)
                rstd_bc = pa.tile([128, K], F32)
                mupad = pa.tile([128, 2, K], F8)
                nc.vector.memset(mupad, 0.0)
                pa_sq = sa.enter_context(tc.tile_pool(name="pa_sq", bufs=2))
                with tc.tile_pool(name="pa_st", bufs=1, space="PSUM") as pa_st:
                    mu_ps = [pa_st.tile([128, 512], F32, tag=f"mu{i}",
                                        name=f"mu{i}") for i in range(KT4)]
                    ms_ps = [pa_st.tile([128, 512], F32, tag=f"ms{i}",
                                        name=f"ms{i}") for i in range(KT4)]
                    for p in range(4):
                        for i in range(KT4):
                            nc.tensor.matmul(
                                mu_ps[i], ones8,
                                mem8_sb[:, 2 * p:2 * p + 2, bass.ts(i, 512)],
                                start=(p == 0), stop=(p == 3), perf_mode=DR)
                    for p in range(4):
                        for i in range(KT4):
                            nc.tensor.matmul(
                                ms_ps[i], ones8,
                                sq8[:, 2 * p:2 * p + 2, bass.ts(i, 512)],
                                start=(p == 0), stop=(p == 3), perf_mode=DR)
                    for i in range(KT4):
                        sl = bass.ts(i, 512)
                        nc.scalar.activation(out=mu_bc[:, sl], in_=mu_ps[i],
                                             func=AF.Copy, scale=1.0 / 1024.0)
                        msq = pa_sq.tile([128, 512], F32, tag="msq",
                                         name="msq")
                        nc.vector.tensor_mul(msq, mu_bc[:, sl], mu_bc[:, sl])
                        var = pa_sq.tile([128, 512], F32, tag="var",
                                         name="var")
                        nc.vector.scalar_tensor_tensor(
                            out=var, in0=ms_ps[i], scalar=1.0 / 1024.0,
                            in1=msq, op0=OP.mult, op1=OP.subtract)
                        nc.scalar.activation(out=var, in_=var, func=AF.Ln,
                                             bias=eps_t, scale=1.0)
                        nc.scalar.activation(out=rstd_bc[:, sl], in_=var,
                                             func=AF.Exp, scale=-0.5)
                # mupad row 0 half 0 = fp8(mu)
                nc.scalar.activation(out=mupad[0:1, 0, :], in_=mu_bc[0:1, :],
                                     func=AF.Copy)

                with ExitStack() as skv:
                    pa_w = skv.enter_context(tc.tile_pool(name="pa_w",
                                                          bufs=3))
                    pa_ps = skv.enter_context(
                        tc.tile_pool(name="pa_ps", bufs=2, space="PSUM"))
                    # K projection -> feature-major K8, rstd folded at drain
                    for oc in range(HC):
                        kps = [pa_ps.tile([128, 512], F32, tag=f"kps{i}",
                                          name=f"kps{i}") for i in range(KT4)]
                        wks = pa_w.tile([128, HC, 128], F8, tag="wk",
                                        name="wks")
                        nc.sync.dma_start(out=wks, in_=d_wk8[oc])
                        for p in range(4):
                            for i in range(KT4):
                                nc.tensor.matmul(
                                    kps[i], wks[:, 2 * p:2 * p + 2, :],
                                    mem8_sb[:, 2 * p:2 * p + 2,
                                            bass.ts(i, 512)],
                                    start=(p == 0), stop=False, perf_mode=DR)
                        for i in range(KT4):
                            nc.tensor.matmul(
                                kps[i],
                                ckpad_sb[:, :, oc * 128:(oc + 1) * 128],
                                mupad[:, :, bass.ts(i, 512)],
                                start=False, stop=True, perf_mode=DR)
                        for i in range(KT4):
                            sl = bass.ts(i, 512)
                            if i < 2:
                                nc.vector.tensor_copy(K8_sb[:, oc, sl],
                                                      kps[i])
                            else:
                                nc.scalar.activation(out=K8_sb[:, oc, sl],
                                                     in_=kps[i], func=AF.Copy)
                    # transposed rstd (per-key partitions) for the V drain
                    for kc in range(KC):
                        tpp = pa_ps.tile([128, 512], F32, tag=f"kps{kc % 2}",
                                         name="tpp")
                        nc.tensor.transpose(
                            tpp[:, 0:128], rstd_bc[:, kc * 128:(kc + 1) * 128],
                            ident)
                        nc.scalar.activation(out=rstd_t[:, kc:kc + 1],
                                             in_=tpp[:, 0:1], func=AF.Copy)
                        nc.scalar.activation(out=rstd16_t[:, kc:kc + 1],
                                             in_=tpp[:, 0:1], func=AF.Copy,
                                             scale=1.0 / 16.0)
                    # V projection -> key-major V8 [128, KC, H]
                    for kc in range(KC):
                        vps = [pa_ps.tile([128, 512], F32, tag=f"kps{2 + ow}",
                                          name=f"vps{ow}")
                               for ow in range(2)]
                        for p in range(4):
                            for ow in range(2):
                                nc.tensor.matmul(
                                    vps[ow],
                                    mem8_sb[:, 2 * p:2 * p + 2,
                                            kc * 128:(kc + 1) * 128],
                                    wvT_sb[:, 2 * p:2 * p + 2,
                                           bass.ts(ow, 512)],
                                    start=(p == 0), stop=False, perf_mode=DR)
                        for ow in range(2):
                            nc.tensor.matmul(
                                vps[ow], mupad[:, :, kc * 128:(kc + 1) * 128],
                                cvpad_sb[:, :, bass.ts(ow, 512)],
                                start=False, stop=True, perf_mode=DR)
                        nc.vector.tensor_scalar_mul(
                            out=V8_sb[:, kc, 0:512], in0=vps[0],
                            scalar1=rstd_t[:, kc:kc + 1])
                        nc.scalar.activation(out=V8_sb[:, kc, 512:1024],
                                             in_=vps[1], func=AF.Copy,
                                             scale=rstd_t[:, kc:kc + 1])

            # xb for the interleaved D1 x-part (DMA lands during late A)
            for hc in range(HC):
                nc.sync.dma_start(out=xb_sb[:, hc, :],
                                  in_=d_xb[hc * 128:(hc + 1) * 128, :])

            # --- C: attention (scalar-bound) interleaved with D1 x-part ---
            with ExitStack() as sc_:
                pc_e = sc_.enter_context(tc.tile_pool(name="pc_e", bufs=1))
                pc_o = sc_.enter_context(tc.tile_pool(name="pc_o", bufs=2))
                pw1x = sc_.enter_context(tc.tile_pool(name="pw1x", bufs=3))
                pc_ps = sc_.enter_context(
                    tc.tile_pool(name="pc_ps", bufs=1, space="PSUM"))
                for h in range(NH):
                    e2 = [[pc_e.tile([128, 2, 512], F8, tag=f"e{tp}_{qt}",
                                     name=f"e{tp}_{qt}") for qt in range(RT)]
                          for tp in range(8)]
                    ctx_ps = [[pc_ps.tile([128, 512], F32, tag=f"ctx{qt}{j}",
                                          name=f"ctx{qt}{j}")
                               for j in range(2)] for qt in range(RT)]
                    for tp in range(8):
                        for ktl in range(2):
                            kt = 2 * tp + ktl
                            scs = [pc_ps.tile([128, 512], F32, tag=f"sc{qt}",
                                              name=f"sc{qt}")
                                   for qt in range(RT)]
                            for qt in range(RT):
                                nc.tensor.matmul(
                                    scs[qt],
                                    K8_sb[:, 2 * h:2 * h + 2,
                                          kt * 128:(kt + 1) * 128],
                                    Q8_sb[:, 2 * h:2 * h + 2,
                                          bass.ts(qt, 512)],
                                    start=True, stop=True, perf_mode=DR)
                            for qt in range(RT):
                                nc.scalar.activation(
                                    out=e2[tp][qt][:, ktl, :], in_=scs[qt],
                                    func=AF.Exp,
                                    scale=rstd16_t[:, kt:kt + 1])
                        for j in range(2):
                            for qt in range(RT):
                                nc.tensor.matmul(
                                    ctx_ps[qt][j],
                                    V8_sb[:, 2 * tp:2 * tp + 2,
                                          h * HD + j * 128:
                                          h * HD + (j + 1) * 128],
                                    e2[tp][qt],
                                    start=(tp == 0), stop=(tp == 7),
                                    perf_mode=DR)
                    # deferred softmax sums (single ones LDW, reuses sc banks)
                    sums_ps = [pc_ps.tile([128, 512], F32, tag=f"sc{qt}",
                                          name=f"sums{qt}")
                               for qt in range(RT)]
                    for tp in range(8):
                        for qt in range(RT):
                            nc.tensor.matmul(
                                sums_ps[qt], ones8, e2[tp][qt],
                                start=(tp == 0), stop=(tp == 7), perf_mode=DR)
                    for qt in range(RT):
                        qsl = bass.ts(qt, 512)
                        rec = pc_o.tile([128, 512], F32, tag="rec",
                                        name="rec")
                        nc.vector.reciprocal_approx_fast(out=rec,
                                                         in_=sums_ps[qt])
                        for j in range(2):
                            tmp = pc_o.tile([128, 512], F32, tag="ctmp",
                                            name="ctmp")
                            nc.vector.tensor_mul(tmp, ctx_ps[qt][j], rec)
                            nc.scalar.activation(
                                out=ctx8_sb[:, 2 * h + j, qsl], in_=tmp,
                                func=AF.Identity,
                                bias=bv_sb[:, 2 * h + j:2 * h + j + 1])
                    # D1 x-part: 4 output chunks per head iteration
                    for oc2 in range(4 * h, 4 * h + 4):
                        w1xs = pw1x.tile([128, HC, 128], BF16, tag="w1x",
                                         name="w1xs")
                        nc.sync.dma_start(out=w1xs, in_=d_w1x[oc2])
                        xps = [pc_ps.tile([128, 512], F32, tag=f"xps{rt}",
                                          name=f"xps{rt}")
                               for rt in range(RT)]
                        for hc in range(HC):
                            for rt in range(RT):
                                nc.tensor.matmul(
                                    xps[rt], w1xs[:, hc, :],
                                    xb_sb[:, hc, bass.ts(rt, 512)],
                                    start=(hc == 0), stop=(hc == HC - 1))
                        for rt in range(RT):
                            nc.vector.tensor_scalar_add(
                                out=h1_sb[:, oc2, bass.ts(rt, 512)],
                                in0=xps[rt],
                                scalar1=b1_sb[:, oc2:oc2 + 1])

        # ================= Phase D: gated integration MLP =================
        l2g_bc = singles.tile([128, H], F32)
        nc.sync.dma_start(
            out=l2g_bc,
            in_=d_l2g[:].unsqueeze(0).partition_broadcast(128).squeeze(1))
        l2b_bc = singles.tile([128, H], F32)
        nc.sync.dma_start(
            out=l2b_bc,
            in_=d_l2b[:].unsqueeze(0).partition_broadcast(128).squeeze(1))

        with ExitStack() as sd:
            pd = sd.enter_context(tc.tile_pool(name="pd", bufs=1))
            x32_sb = pd.tile([128, HC, R], F32)    # 4 MB (used in D3)
            pd_wd3 = sd.enter_context(tc.tile_pool(name="pd_wd3", bufs=3))
            pd_y = sd.enter_context(tc.tile_pool(name="pd_y", bufs=1))
            yt_sb = pd_y.tile([128, HC, R], F32)
            pd_o = sd.enter_context(tc.tile_pool(name="pd_o", bufs=2))
            pd_yr = sd.enter_context(tc.tile_pool(name="pd_yr", bufs=4))

            pd_st = sd.enter_context(tc.tile_pool(name="pd_st", bufs=1))
            mu2_sb = pd_st.tile([128, R], F32)
            rstd2_sb = pd_st.tile([128, R], F32)
            pd_sq = sd.enter_context(tc.tile_pool(name="pd_sq", bufs=2))
            pd_w1 = sd.enter_context(tc.tile_pool(name="pd_w1", bufs=3))
            pd_ps = sd.enter_context(
                tc.tile_pool(name="pd_ps", bufs=2, space="PSUM"))
            pd_ps2 = sd.enter_context(
                tc.tile_pool(name="pd_ps2", bufs=1, space="PSUM"))
            mu2_ps = [pd_ps2.tile([128, 512], F32, tag=f"m2_{i}",
                                  name=f"m2_{i}") for i in range(RT)]
            ms2_ps = [pd_ps2.tile([128, 512], F32, tag=f"s2_{i}",
                                  name=f"s2_{i}") for i in range(RT)]
            sq2s = [None] * H2C

            def d2_stats(oc2):
                for rt in range(RT):
                    sl = bass.ts(rt, 512)
                    nc.tensor.matmul(mu2_ps[rt], ones_bf,
                                     h1_sb[:, oc2, sl],
                                     start=(oc2 == 0),
                                     stop=(oc2 == H2C - 1))
                    nc.tensor.matmul(ms2_ps[rt], ones_bf,
                                     sq2s[oc2][:, sl],
                                     start=(oc2 == 0),
                                     stop=(oc2 == H2C - 1))

            # D1 remainder: h1 += ctx-part (fp8 DR); D2 stats lag-3 pipelined;
            # D3 weight + x32 DMAs spread through the loop
            for oc2 in range(H2C):
                w1as = pd_w1.tile([128, HC, 128], F8, tag="w1a", name="w1as")
                nc.sync.dma_start(out=w1as, in_=d_w1a[oc2])
                if oc2 % 2 == 0:
                    hc = oc2 // 2
                    nc.sync.dma_start(out=x32_sb[:, hc, :],
                                      in_=d_x32[hc * 128:(hc + 1) * 128, :])
                cps = [pd_ps.tile([128, 512], F32, tag=f"cps{rt}",
                                  name=f"cps{rt}") for rt in range(RT)]
                for p in range(4):
                    for rt in range(RT):
                        nc.tensor.matmul(
                            cps[rt], w1as[:, 2 * p:2 * p + 2, :],
                            ctx8_sb[:, 2 * p:2 * p + 2, bass.ts(rt, 512)],
                            start=(p == 0), stop=(p == 3), perf_mode=DR)
                for rt in range(RT):
                    sl = bass.ts(rt, 512)
                    nc.vector.tensor_add(h1_sb[:, oc2, sl], cps[rt],
                                         h1_sb[:, oc2, sl])
                sq2 = pd_sq.tile([128, R], BF16, tag="sq2", name="sq2",
                                 bufs=4)
                nc.scalar.activation(out=sq2, in_=h1_sb[:, oc2, :],
                                     func=AF.Square)
                sq2s[oc2] = sq2
                if oc2 >= 3:
                    d2_stats(oc2 - 3)
            for oc2 in range(H2C - 3, H2C):
                d2_stats(oc2)

            # D2 drains: mu2 / rstd2
            for rt in range(RT):
                sl = bass.ts(rt, 512)
                nc.scalar.activation(out=mu2_sb[:, sl], in_=mu2_ps[rt],
                                     func=AF.Copy, scale=1.0 / 2048.0)
                msq = pd_sq.tile([128, 512], F32, tag="msq2", name="msq2")
                nc.vector.tensor_mul(msq, mu2_sb[:, sl], mu2_sb[:, sl])
                var = pd_sq.tile([128, 512], F32, tag="var2", name="var2")
                nc.vector.scalar_tensor_tensor(
                    out=var, in0=ms2_ps[rt], scalar=1.0 / 2048.0,
                    in1=msq, op0=OP.mult, op1=OP.subtract)
                nc.scalar.activation(out=var, in_=var, func=AF.Ln,
                                     bias=eps_t, scale=1.0)
                nc.scalar.activation(out=rstd2_sb[:, sl], in_=var,
                                     func=AF.Exp, scale=-0.5)

            def d2_apply(rt):
                sl = bass.ts(rt, 512)
                for oc2 in range(H2C):
                    t1 = pd_sq.tile([128, 512], F32, tag="t1d", name="t1d")
                    nc.vector.tensor_sub(t1, h1_sb[:, oc2, sl], mu2_sb[:, sl])
                    nc.vector.scalar_tensor_tensor(
                        out=t1, in0=t1, scalar=ilg_sb[:, oc2:oc2 + 1],
                        in1=rstd2_sb[:, sl], op0=OP.mult, op1=OP.mult)
                    nc.scalar.activation(out=h1_sb[:, oc2, sl], in_=t1,
                                         func=AF.Gelu,
                                         bias=ilb_sb[:, oc2:oc2 + 1])

            def cat8_pair(p, sl):
                if p < 4:
                    return x8_sb[:, 2 * p:2 * p + 2, sl]
                return ctx8_sb[:, 2 * (p - 4):2 * (p - 4) + 2, sl]

            all_sigs = {}

            def gates(rt):
                sl = bass.ts(rt, 512)
                sigs = []
                for oc in range(HC):
                    gws = pd_wd3.tile([128, H2C, 128], F8, tag="gw",
                                      name="gws")
                    nc.sync.dma_start(out=gws, in_=d_gw8[oc])
                    gps = pd_ps.tile([128, 512], F32, tag="cps0", name="gps")
                    for p in range(8):
                        nc.tensor.matmul(
                            gps, gws[:, 2 * p:2 * p + 2, :],
                            cat8_pair(p, sl),
                            start=(p == 0), stop=(p == 7), perf_mode=DR)
                    sig = pd_o.tile([128, 512], BF16, tag="sig", name="sig",
                                    bufs=8)
                    nc.scalar.activation(out=sig, in_=gps, func=AF.Sigmoid,
                                         bias=gb_sb[:, oc:oc + 1])
                    sigs.append(sig)
                all_sigs[rt] = sigs

            def integs(rt):
                sl = bass.ts(rt, 512)
                for oc in range(HC):
                    w2s = pd_wd3.tile([128, H2C, 128], BF16, tag="w2",
                                      name="w2s")
                    nc.sync.dma_start(out=w2s, in_=d_w2b[oc])
                    igps = pd_ps.tile([128, 512], F32, tag="cps1",
                                      name="igps")
                    for hc in range(H2C):
                        nc.tensor.matmul(igps, w2s[:, hc, :],
                                         h1_sb[:, hc, sl],
                                         start=(hc == 0),
                                         stop=(hc == H2C - 1))
                    tmp = pd_o.tile([128, 512], F32, tag="ytmp", name="ytmp")
                    nc.vector.scalar_tensor_tensor(
                        out=tmp, in0=igps, scalar=b2_sb[:, oc:oc + 1],
                        in1=all_sigs[rt][oc], op0=OP.add, op1=OP.mult)
                    nc.vector.tensor_add(yt_sb[:, oc, sl], tmp,
                                         x32_sb[:, oc, sl])

            def d4_pass(rt):
                mvs = pd_o.tile([128, 4, 2], F32, tag="mvs", name="mvs")
                yrs = []
                for i, rc in enumerate(range(rt * 4, rt * 4 + 4)):
                    tps = [pd_ps2.tile([128, 512], F32,
                                       tag=(f"m2_{k}" if i % 2 == 0
                                            else f"s2_{k}"),
                                       name=f"tp{k}") for k in range(2)]
                    for oc in range(HC):
                        nc.tensor.transpose(
                            tps[oc // 4][:, (oc % 4) * 128:
                                         (oc % 4 + 1) * 128],
                            yt_sb[:, oc, rc * 128:(rc + 1) * 128], ident)
                    yr = pd_yr.tile([128, H], F32, tag="yr", name="yr")
                    nc.scalar.activation(out=yr[:, 0:512], in_=tps[0],
                                         func=AF.Copy)
                    nc.scalar.activation(out=yr[:, 512:1024], in_=tps[1],
                                         func=AF.Copy)
                    stats = pd_o.tile([128, 2, 6], F32, tag="bst", name="bst")
                    for k in range(2):
                        nc.vector.bn_stats(out=stats[:, k, :],
                                           in_=yr[:, k * 512:(k + 1) * 512])
                    nc.vector.bn_aggr(out=mvs[:, i, :], in_=stats)
                    yrs.append(yr)
                rstds = pd_o.tile([128, 4], F32, tag="rstds", name="rstds")
                nc.scalar.activation(out=rstds, in_=mvs[:, :, 1],
                                     func=AF.Ln, bias=eps_t, scale=1.0)
                nc.scalar.activation(out=rstds, in_=rstds,
                                     func=AF.Exp, scale=-0.5)
                nmrs = pd_o.tile([128, 4], F32, tag="nmrs", name="nmrs")
                nc.vector.scalar_tensor_tensor(
                    out=nmrs, in0=mvs[:, :, 0], scalar=-1.0, in1=rstds,
                    op0=OP.mult, op1=OP.mult)
                for i, rc in enumerate(range(rt * 4, rt * 4 + 4)):
                    yr = yrs[i]
                    nc.scalar.activation(out=yr, in_=yr, func=AF.Identity,
                                         bias=nmrs[:, i:i + 1],
                                         scale=rstds[:, i:i + 1])
                    nc.vector.tensor_mul(yr, yr, l2g_bc)
                    nc.vector.tensor_add(yr, yr, l2b_bc)
                    nc.sync.dma_start(
                        out=d_out[rc * 128:(rc + 1) * 128, :], in_=yr)

            d2_apply(0)
            gates(0)
            d2_apply(1)
            integs(0)
            gates(1)
            d4_pass(0)
            integs(1)
            d4_pass(1)

    nc.compile()
    return nc


_NC_CACHE = []


def _get_nc():
    if not _NC_CACHE:
        _NC_CACHE.append(build_program())
    return _NC_CACHE[0]


def kernel(query_hidden, mem_keys, importance, recency, access_count,
           Wq, bq, in_w, in_b, out_w, out_b, gate_w, gate_b,
           int_w1, int_b1, int_ln_g, int_ln_b, int_w2, int_b2,
           ln1_g, ln1_b, ln2_g, ln2_b, sel_params, top_k):
    np32 = lambda a: np.asarray(a, dtype=np.float32)
    f8 = lambda a: np.ascontiguousarray(np.asarray(a, np.float32).astype(NP_F8))
    bf = lambda a: np.ascontiguousarray(np.asarray(a, np.float32).astype(NP_BF16))
    query_hidden = np32(query_hidden)
    mem_keys = np32(mem_keys)
    top_k = int(top_k)
    assert top_k == K, f"kernel compiled for top_k={K}, got {top_k}"

    # HTPS selection (host): softmax-weighted score, top-k set, gather.
    # Attention output is invariant to the order of the selected rows.
    sp = np32(sel_params)
    w = np.exp(sp - sp.max())
    w = w / w.sum()
    acc = np32(access_count)
    sel = w[0] * np32(importance) + w[1] * np32(recency) + w[2] * (acc / acc.max())
    idx = np.argpartition(-sel, top_k - 1)[:top_k]
    mem = mem_keys[idx]                                 # [K, H]

    in_w = np32(in_w)
    in_b = np32(in_b)
    wq, wk, wv = in_w[:H], in_w[H:2 * H], in_w[2 * H:]
    bqi, bki, bvi = in_b[:H], in_b[H:2 * H], in_b[2 * H:]
    wc = wq @ np32(Wq)                                  # fused query projection
    bc = wq @ np32(bq) + bqi

    # fold mem-layernorm gamma into wk/wv; beta into the biases.
    # bk is softmax-invariant -> dropped. bv applied to ctx post-softmax.
    g1 = np32(ln1_g)
    b1v = np32(ln1_b)
    bv_f = bvi + wv @ b1v
    wk_f = wk * g1[None, :]
    wv_f = wv * g1[None, :]
    ck = wk_f.sum(axis=1)                               # [H]
    cv = wv_f.sum(axis=1)

    ckpad = np.zeros((128, 2, H), NP_F8)
    ckpad[0, 0, :] = (-ck).astype(NP_F8)
    cvpad = np.zeros((128, 2, H), NP_F8)
    cvpad[0, 0, :] = (-cv).astype(NP_F8)

    # fold attn_out = ctx @ out_w.T + out_b into gate / integration weights
    out_w = np32(out_w)
    out_b = np32(out_b)
    gate_w = np32(gate_w)
    int_w1 = np32(int_w1)
    gwx, gwa = gate_w[:, :H], gate_w[:, H:]
    w1x, w1a = int_w1[:, :H], int_w1[:, H:]
    gate_b_f = np32(gate_b) + gwa @ out_b
    int_b1_f = np32(int_b1) + w1a @ out_b

    T = lambda a: np.ascontiguousarray(np32(a).T)

    def chunked(w_t, dt):
        # [IN, OUT] -> [OUT//128, 128, IN//128, 128] contiguous slabs
        inn, out = w_t.shape
        r = w_t.reshape(inn // 128, 128, out // 128, 128).transpose(2, 1, 0, 3)
        return np.ascontiguousarray(r.astype(dt))

    gw_full = np.concatenate([gwx.T, (gwa @ out_w).T], axis=0)   # [2H, H]
    wvT = np.ascontiguousarray(
        wv_f.T.reshape(HC, 128, H).transpose(1, 0, 2))           # [128, HC, H]

    common = {
        "mem8_t": f8(mem.T),
        "wk8": chunked(T(wk_f), NP_F8),
        "wc8": chunked(T(wc), NP_F8),
        "wvT8": wvT.astype(NP_F8),
        "ckpad": ckpad, "cvpad": cvpad,
        "gw8": chunked(gw_full, NP_F8),
        "w1x": chunked(T(w1x), NP_BF16),
        "w1a": chunked(T(w1a @ out_w), NP_F8),
        "w2b": chunked(T(np32(int_w2)), NP_BF16),
        "bc": bc, "bv": bv_f,
        "gate_b": gate_b_f, "int_b1": int_b1_f, "int_b2": np32(int_b2),
        "iln_g": np32(int_ln_g), "iln_b": np32(int_ln_b),
        "ln2_g": np32(ln2_g), "ln2_b": np32(ln2_b),
    }
    X = query_hidden.reshape(B * S, H)
    in_maps = []
    for c in range(N_CORES):
        xt = np.ascontiguousarray(X[c * R:(c + 1) * R].T)
        m = dict(common)
        m["x32_t"] = xt
        m["x8_t"] = f8(xt)
        m["xb_t"] = bf(xt)
        in_maps.append(m)

    nc = _get_nc()
    res = run_bass_kernel_spmd(nc, in_maps, core_ids=list(range(N_CORES)))
    out = np.empty((B * S, H), dtype=np.float32)
    for c in range(N_CORES):
        out[c * R:(c + 1) * R] = res.results[c]["out"]
    return out.reshape(B, S, H)
